# revision 70
# baseline (speedup 1.0000x reference)
"""Trainium2 Bass kernel for windowed (local) causal self-attention.

Reference computation (per batch element, fp32):
    q = x @ Wq.T + bq ; k = x @ Wk.T + bk ; v = x @ Wv.T + bv
    per non-overlapping window of 256 tokens:
        attn = softmax(causal_mask(q k^T * HEAD_DIM**-0.5))
        out  = attn @ v
    o = out @ Wo.T + bo + x

Algebraic restructure (no head split in this module, softmax rows sum to 1):
    scores = q k^T = x M x^T + cq 1^T + 1 ck^T + bq.bk,  M  = Wq^T Wk
        cq = x (Wq^T bk)  [per-QUERY shift: cancels in softmax, dropped]
        ck = x (Wk^T bq)  [per-KEY: folded into the ACT exp bias]
    o = attn (x N) + (bv Wo^T + bo) + x,      N  = Wv^T Wo^T
so only TWO E x E projections remain on device (q' = x M and v' = x N);
M, N, ck and the constant output row are computed on the host in float64.
The residual + constant row are also added on the host.

Sharding: data-parallel over (batch, window): 64 window-blocks of 256
tokens -> 8 cores x 8 windows.  M, N replicated.

Per-core kernel strategy:
  - matmul OPERANDS (M, N, x, q'T) are bf16 (same PE rate as fp32r but
    half the DMA/SBUF/weight-load traffic); every accumulation and the
    attention core (exp, attn weights, v', out) stay fp32.
  - scores are computed TRANSPOSED, sT[k, q] = x_k . q'_q, so no PE
    transposes of the attention matrix are needed: exp(sT) chunks serve
    directly as the stationary operand of out = attn @ v'.
  - causal block-sparsity: the kt=1 key block only serves queries q>=128,
    so its score matmuls/exp narrow to 128 columns and the (qt=0, kt=1)
    output/sum matmuls are skipped.
  - softmax row sums become N=2 matmuls (expT^T @ ones2) accumulated over
    k-chunks; normalization is folded into the ACT output evacuation as a
    per-partition scale (1/sum).
  - q'-projection is window-PAIRED (moving 512 tokens) to halve its
    instruction count and PE weight-load switches.
  - v' is computed token-major between the score matmuls and the
    attention matmuls so the PE stays busy through the softmax chain.
  - window-0 M/N DMA is chunked and interleaved with compute emission,
    with tiny PE warmup matmuls paced by arriving chunks; xT loads are
    prefetched one pair ahead; output stores are deferred behind the
    next pair's loads.
"""
import sys

sys.path.insert(0, "/opt/trn_rl_repo")

import numpy as np
import ml_dtypes

import concourse.bass as bass
import concourse.bacc as bacc
import concourse.mybir as mybir
import concourse.tile as tile
from concourse.bass_utils import run_bass_kernel_spmd

F32 = mybir.dt.float32
F32R = mybir.dt.float32r
BF16 = mybir.dt.bfloat16
NP_BF16 = ml_dtypes.bfloat16
AF = mybir.ActivationFunctionType

E = 1024          # embed dim
ET = E // 128     # e-tiles
W = 256           # window size
NW = 8            # windows per core
T = NW * W        # tokens per core
N_CORES = 8
SCALE = (E // 16) ** (-0.5)  # HEAD_DIM ** -0.5 = 0.125
NEG = -1.0e30
PW = 2 * W        # tokens per window pair


def build_nc(nw=NW):
    t_core = nw * W
    npair = nw // 2
    nc = bacc.Bacc("TRN2", target_bir_lowering=False, debug=False)

    xt_d = nc.dram_tensor("xt", [E, t_core], BF16, kind="ExternalInput")
    m_d = nc.dram_tensor("m", [E, E], BF16, kind="ExternalInput")
    n_d = nc.dram_tensor("n", [E, E], BF16, kind="ExternalInput")
    # ck * SCALE laid out as one [128] column per 128-token chunk
    ckc_d = nc.dram_tensor("ckc", [128, 2 * nw], F32, kind="ExternalInput")
    o_d = nc.dram_tensor("o", [t_core, E], F32, kind="ExternalOutput")

    # transposed causal mask constants applied to sT[k, q] = score(q, k).
    # kt=0: full [k, 0:256] triangle.  kt=1: only queries q>=128 are kept
    # downstream, stored in columns 0:128 (q = 128 + col).  (The bq.bk
    # score constant and the per-query cq row are dropped: uniform per-row
    # logit shifts cancel in softmax.)
    mask_np = np.full((2, 128, W), NEG, dtype=np.float32)
    k_idx = np.arange(128)[:, None]
    mask_np[0][k_idx <= np.arange(W)[None, :]] = 0.0
    mask_np[1][:, 0:128][k_idx <= np.arange(128)[None, :]] = 0.0
    mask_d = nc.inline_tensor(mask_np, "mask")
    # two identical ones-columns: fp32r matmul dst free size must be even,
    # so the softmax row sums are computed as N=2 (duplicate) columns
    onec_d = nc.inline_tensor(np.ones((128, 2), dtype=np.float32), "onec")

    with tile.TileContext(nc) as tc:
        with (
            tc.tile_pool(name="wp", bufs=1) as wp,
            tc.tile_pool(name="cp", bufs=1) as cp,
            tc.tile_pool(name="xtp", bufs=2) as xtp,
            tc.tile_pool(name="qtp", bufs=2) as qtp,
            tc.tile_pool(name="etp", bufs=2) as etp,
            tc.tile_pool(name="sp", bufs=4) as sp,
            tc.tile_pool(name="vp", bufs=4) as vp,
            tc.tile_pool(name="smp", bufs=8) as smp,
            tc.tile_pool(name="op", bufs=4) as op,
            tc.tile_pool(name="ps_qk", bufs=3, space=bass.MemorySpace.PSUM) as ps_qk,
            tc.tile_pool(name="ps_big", bufs=3, space=bass.MemorySpace.PSUM) as ps_big,
            # 2 bufs so the qt1 row-sum matmul never waits on the DVE
            # reciprocal still reading qt0's psum; warms share the same
            # tag/shape so no extra bank is needed
            tc.tile_pool(name="ps_sm", bufs=2, space=bass.MemorySpace.PSUM) as ps_sm,
        ):
            # ---- resident constants (loaded later, on the sync queue, so
            # no second DMA ring has to be initialized at boot; they are
            # first needed ~16us in, well after the weight chunks) ----
            masks = cp.tile([128, 2, W], F32, tag="mask")
            onec = cp.tile([128, 2], F32R, tag="onec")
            ckc = cp.tile([128, 2 * nw], F32, tag="ckc")

            def load_consts():
                for kt in range(2):
                    nc.sync.dma_start(masks[:, kt, :], mask_d.ap()[kt])
                nc.sync.dma_start(onec[:], onec_d.ap().bitcast(F32R))
                nc.sync.dma_start(ckc[:], ckc_d.ap())

            # ---- resident weights: [p, ei, eo] = Wmat[ei*128+p, eo] ----
            msb = wp.tile([128, ET, E], BF16, tag="m", name="msb")
            nsb = wp.tile([128, ET, E], BF16, tag="n", name="nsb")
            m_r = m_d.ap().rearrange("(a p) n -> a p n", p=128)
            n_r = n_d.ap().rearrange("(a p) n -> a p n", p=128)

            def warm(col=0):
                # keep the PE activity monitor warm through the DMA-bound
                # phase: a tiny matmul per arriving chunk, paced by the DMA
                # itself (reads a slice of the chunk that just landed)
                wps = ps_sm.tile([128, 2], F32, tag="sum", name="warm")
                nc.tensor.matmul(
                    wps[:],
                    msb[:, 0, col : col + 128],
                    msb[:, 0, col : col + 2],
                    start=True,
                    stop=True,
                )

            xtr = xt_d.ap().rearrange("(a p) t -> a p t", p=128)
            xT_next = None
            for p in range(npair):
                ptok0 = p * PW

                # ---- xT[p, ei, t] for the pair (e-major, host-transposed) ----
                if p == 0:
                    xT = xtp.tile([128, ET, PW], BF16, tag="xT")
                    # interleave M column-chunks with per-ei chunks of the
                    # pair-0 xT load so the first q'-proj matmuls start as
                    # soon as (xT ei 0-1, M chunk 0) land
                    for eo in range(ET):
                        if eo < 4:
                            nc.sync.dma_start(
                                xT[:, 2 * eo : 2 * eo + 2, :],
                                xtr[2 * eo : 2 * eo + 2, :, 0:PW].transpose(
                                    [1, 0, 2]
                                ),
                            )
                        nc.sync.dma_start(
                            msb[:, :, eo * 128 : (eo + 1) * 128],
                            m_r[:, :, eo * 128 : (eo + 1) * 128].transpose([1, 0, 2]),
                        )
                        warm(eo * 128)
                    load_consts()
                else:
                    xT = xT_next

                # ---- q' projection for the pair -> q'T [e_out, t(512)] ----
                qT = qtp.tile([128, ET, PW], BF16, tag="qT")
                for eo in range(ET):
                    pp = ps_big.tile([128, PW], F32, tag="big")
                    for ei in range(ET):
                        nc.tensor.matmul(
                            pp[:],
                            msb[:, ei, eo * 128 : (eo + 1) * 128],
                            xT[:, ei, :],
                            start=(ei == 0),
                            stop=(ei == ET - 1),
                        )
                    nc.scalar.copy(qT[:, eo, :], pp[:])

                if p == 0:
                    # N chunk loads slot in behind the pair-0 q' matmuls
                    for half in range(2):
                        for eq in range(0, ET, 4):
                            nc.sync.dma_start(
                                nsb[:, eq : eq + 4, half * 512 : (half + 1) * 512],
                                n_r[
                                    eq : eq + 4, :, half * 512 : (half + 1) * 512
                                ].transpose([1, 0, 2]),
                            )

                # prefetch next pair's xT behind this pair's compute
                if p + 1 < npair:
                    xT_next = xtp.tile([128, ET, PW], BF16, tag="xT")
                    nc.sync.dma_start(
                        xT_next[:, :, :],
                        xtr[:, :, ptok0 + PW : ptok0 + 2 * PW].transpose([1, 0, 2]),
                    )

                for wi in range(2):
                    w = 2 * p + wi
                    tok0 = w * W
                    wt0 = wi * W  # token offset inside the pair tiles

                    # ---- transposed scores sT[k, q] + softmax ----
                    # causal: the kt=1 key block only serves queries q>=128;
                    # its block is computed 128 columns wide (q = 128+col).
                    expT = etp.tile([128, 2, W], F32R, tag="expT")
                    for kt in range(2):
                        qw = W if kt == 0 else 128
                        q0 = wt0 + kt * 128  # first query column needed
                        sc = ps_qk.tile([128, W], F32, tag="qk")
                        for ei in range(ET):
                            nc.tensor.matmul(
                                sc[:, 0:qw],
                                xT[:, ei, wt0 + kt * 128 : wt0 + (kt + 1) * 128],
                                qT[:, ei, q0 : q0 + qw],
                                start=(ei == 0),
                                stop=(ei == ET - 1),
                            )
                        s_sb = sp.tile([128, W], F32, tag="s")
                        nc.vector.tensor_add(
                            s_sb[:, 0:qw], sc[:, 0:qw], masks[:, kt, 0:qw]
                        )
                        # exp(SCALE*s + SCALE*ck[k]): per-key bias via ACT
                        nc.scalar.activation(
                            expT[:, kt, kt * 128 : kt * 128 + qw],
                            s_sb[:, 0:qw],
                            AF.Exp,
                            scale=SCALE,
                            bias=ckc[:, 2 * w + kt : 2 * w + kt + 1],
                        )

                    # ---- v' projection (token-major), fills PE during softmax ----
                    v_w = [
                        vp.tile([128, E], F32R, tag="v", name=f"v{kt}")
                        for kt in range(2)
                    ]
                    for kt in range(2):
                        for eoh in range(2):
                            pv = ps_big.tile([128, 512], F32, tag="big")
                            for ei in range(ET):
                                nc.tensor.matmul(
                                    pv[:],
                                    xT[:, ei, wt0 + kt * 128 : wt0 + (kt + 1) * 128],
                                    nsb[:, ei, eoh * 512 : (eoh + 1) * 512],
                                    start=(ei == 0),
                                    stop=(ei == ET - 1),
                                )
                            nc.vector.tensor_copy(
                                v_w[kt][:, eoh * 512 : (eoh + 1) * 512], pv[:]
                            )

                    # ---- softmax row sums (over k = partitions) ----
                    # qt=0 queries only attend to kt=0 keys (causal)
                    recs = []
                    for qt in range(2):
                        kts = (0,) if qt == 0 else (0, 1)
                        sm = ps_sm.tile([128, 2], F32, tag="sum")
                        for kt in kts:
                            nc.tensor.matmul(
                                sm[:],
                                expT[:, kt, qt * 128 : (qt + 1) * 128],
                                onec[:],
                                start=(kt == kts[0]),
                                stop=(kt == kts[-1]),
                            )
                        rec = smp.tile([128, 1], F32, tag="rec")
                        nc.vector.reciprocal(rec[:], sm[:, 0:1])
                        recs.append(rec)

                    # ---- out = attn @ v' (token-major), normalize in evac ----
                    # final window runs qt1 first so its stores overlap
                    # qt0's shorter single-block chain at the tail
                    for qt in (1, 0) if w == nw - 1 else (0, 1):
                        kts = (0,) if qt == 0 else (0, 1)
                        o_sb = op.tile([128, E], F32, tag="o")
                        for eoh in range(2):
                            po = ps_big.tile([128, 512], F32, tag="big")
                            for kt in kts:
                                nc.tensor.matmul(
                                    po[:],
                                    expT[:, kt, qt * 128 : (qt + 1) * 128],
                                    v_w[kt][:, eoh * 512 : (eoh + 1) * 512],
                                    start=(kt == kts[0]),
                                    stop=(kt == kts[-1]),
                                )
                            # normalize-evacuate on ACT; for the final
                            # window qt1 goes to the (now idle) DVE so both
                            # engines drain the tail concurrently
                            if qt == 0 or w < nw - 1:
                                nc.scalar.activation(
                                    o_sb[:, eoh * 512 : (eoh + 1) * 512],
                                    po[:],
                                    AF.Copy,
                                    scale=recs[qt][:],
                                )
                            else:
                                nc.vector.tensor_scalar_mul(
                                    o_sb[:, eoh * 512 : (eoh + 1) * 512],
                                    po[:],
                                    recs[qt][:],
                                )
                            # stream the final window's halves out as soon
                            # as each is evacuated; earlier windows store
                            # as one contiguous tile (fewer descriptors).
                            # Stores are emitted after the next pair's
                            # prefetch in sync-queue order, so they never
                            # head-of-line-block it.
                            if w == nw - 1:
                                nc.sync.dma_start(
                                    o_d.ap()[
                                        tok0 + qt * 128 : tok0 + (qt + 1) * 128,
                                        eoh * 512 : (eoh + 1) * 512,
                                    ],
                                    o_sb[:, eoh * 512 : (eoh + 1) * 512],
                                )
                        if w < nw - 1:
                            nc.sync.dma_start(
                                o_d.ap()[tok0 + qt * 128 : tok0 + (qt + 1) * 128, :],
                                o_sb[:],
                            )

    nc.compile()
    return nc


_NC_CACHE = {}


def _get_nc(nw=NW):
    if nw not in _NC_CACHE:
        _NC_CACHE[nw] = build_nc(nw)
    return _NC_CACHE[nw]


def prepare(x, Wq, bq, Wk, bk, Wv, bv, Wo, bo):
    """Host-side precompute: per-core input maps + host residual terms."""
    x = np.asarray(x, dtype=np.float32)
    B, S, _ = x.shape
    x_flat = np.ascontiguousarray(x.reshape(B * S, E))
    t_core = B * S // N_CORES
    assert t_core == T

    f64 = np.float64
    Wq64, Wk64 = np.asarray(Wq, f64), np.asarray(Wk, f64)
    Wv64, Wo64 = np.asarray(Wv, f64), np.asarray(Wo, f64)
    bq64, bk64 = np.asarray(bq, f64), np.asarray(bk, f64)
    bv64, bo64 = np.asarray(bv, f64), np.asarray(bo, f64)

    M = np.ascontiguousarray((Wq64.T @ Wk64).astype(NP_BF16))
    N = np.ascontiguousarray((Wv64.T @ Wo64.T).astype(NP_BF16))
    ck = (x_flat.astype(f64) @ (Wk64.T @ bq64)) * SCALE  # [T_total]
    orow = (bv64 @ Wo64.T + bo64).astype(np.float32)  # [E]
    xt_full = x_flat.T.astype(NP_BF16)

    common = {"m": M, "n": N}
    in_maps = [
        {
            "xt": np.ascontiguousarray(xt_full[:, i * t_core : (i + 1) * t_core]),
            # ck columns: [128, 2*nw], one column per 128-token chunk
            "ckc": np.ascontiguousarray(
                ck[i * t_core : (i + 1) * t_core]
                .astype(np.float32)
                .reshape(2 * NW, 128)
                .T
            ),
            **common,
        }
        for i in range(N_CORES)
    ]
    return in_maps, orow, x_flat, (B, S)


def kernel(x, Wq, bq, Wk, bk, Wv, bv, Wo, bo):
    in_maps, orow, x_flat, (B, S) = prepare(x, Wq, bq, Wk, bk, Wv, bv, Wo, bo)
    nc = _get_nc()
    res = run_bass_kernel_spmd(nc, in_maps, core_ids=list(range(N_CORES)))
    out = np.concatenate([res.results[i]["o"] for i in range(N_CORES)], axis=0)
    out += orow[None, :]
    out += x_flat
    return out.reshape(B, S, E).astype(np.float32)


# revision 71
# speedup vs baseline: 1.0027x; 1.0027x over previous
"""Trainium2 Bass kernel for windowed (local) causal self-attention.

Reference computation (per batch element, fp32):
    q = x @ Wq.T + bq ; k = x @ Wk.T + bk ; v = x @ Wv.T + bv
    per non-overlapping window of 256 tokens:
        attn = softmax(causal_mask(q k^T * HEAD_DIM**-0.5))
        out  = attn @ v
    o = out @ Wo.T + bo + x

Algebraic restructure (no head split in this module, softmax rows sum to 1):
    scores = q k^T = x M x^T + cq 1^T + 1 ck^T + bq.bk,  M  = Wq^T Wk
        cq = x (Wq^T bk)  [per-QUERY shift: cancels in softmax, dropped]
        ck = x (Wk^T bq)  [per-KEY: folded into the ACT exp bias]
    o = attn (x N) + (bv Wo^T + bo) + x,      N  = Wv^T Wo^T
so only TWO E x E projections remain on device (q' = x M and v' = x N);
M, N, ck and the constant output row are computed on the host in float64.
The residual + constant row are also added on the host.

Sharding: data-parallel over (batch, window): 64 window-blocks of 256
tokens -> 8 cores x 8 windows.  M, N replicated.

Per-core kernel strategy:
  - matmul OPERANDS (M, N, x, q'T) are bf16 (same PE rate as fp32r but
    half the DMA/SBUF/weight-load traffic); every accumulation and the
    attention core (exp, attn weights, v', out) stay fp32.
  - scores are computed TRANSPOSED, sT[k, q] = x_k . q'_q, so no PE
    transposes of the attention matrix are needed: exp(sT) chunks serve
    directly as the stationary operand of out = attn @ v'.
  - causal block-sparsity: the kt=1 key block only serves queries q>=128,
    so its score matmuls/exp narrow to 128 columns and the (qt=0, kt=1)
    output/sum matmuls are skipped.
  - softmax row sums become N=2 matmuls (expT^T @ ones2) accumulated over
    k-chunks; normalization is folded into the ACT output evacuation as a
    per-partition scale (1/sum).
  - q'-projection is window-PAIRED (moving 512 tokens) to halve its
    instruction count and PE weight-load switches.
  - v' is computed token-major between the score matmuls and the
    attention matmuls so the PE stays busy through the softmax chain.
  - window-0 M/N DMA is chunked and interleaved with compute emission,
    with tiny PE warmup matmuls paced by arriving chunks; xT loads are
    prefetched one pair ahead; output stores are deferred behind the
    next pair's loads.
"""
import sys

sys.path.insert(0, "/opt/trn_rl_repo")

import numpy as np
import ml_dtypes

import concourse.bass as bass
import concourse.bacc as bacc
import concourse.mybir as mybir
import concourse.tile as tile
from concourse.bass_utils import run_bass_kernel_spmd

F32 = mybir.dt.float32
F32R = mybir.dt.float32r
BF16 = mybir.dt.bfloat16
NP_BF16 = ml_dtypes.bfloat16
AF = mybir.ActivationFunctionType

E = 1024          # embed dim
ET = E // 128     # e-tiles
W = 256           # window size
NW = 8            # windows per core
T = NW * W        # tokens per core
N_CORES = 8
SCALE = (E // 16) ** (-0.5)  # HEAD_DIM ** -0.5 = 0.125
NEG = -1.0e30
PW = 2 * W        # tokens per window pair


def build_nc(nw=NW):
    t_core = nw * W
    npair = nw // 2
    nc = bacc.Bacc("TRN2", target_bir_lowering=False, debug=False)

    xt_d = nc.dram_tensor("xt", [E, t_core], BF16, kind="ExternalInput")
    m_d = nc.dram_tensor("m", [E, E], BF16, kind="ExternalInput")
    n_d = nc.dram_tensor("n", [E, E], BF16, kind="ExternalInput")
    # ck * SCALE laid out as one [128] column per 128-token chunk
    ckc_d = nc.dram_tensor("ckc", [128, 2 * nw], F32, kind="ExternalInput")
    o_d = nc.dram_tensor("o", [t_core, E], F32, kind="ExternalOutput")

    # transposed causal mask constants applied to sT[k, q] = score(q, k).
    # kt=0: full [k, 0:256] triangle.  kt=1: only queries q>=128 are kept
    # downstream, stored in columns 0:128 (q = 128 + col).  (The bq.bk
    # score constant and the per-query cq row are dropped: uniform per-row
    # logit shifts cancel in softmax.)
    mask_np = np.full((2, 128, W), NEG, dtype=np.float32)
    k_idx = np.arange(128)[:, None]
    mask_np[0][k_idx <= np.arange(W)[None, :]] = 0.0
    mask_np[1][:, 0:128][k_idx <= np.arange(128)[None, :]] = 0.0
    mask_d = nc.inline_tensor(mask_np, "mask")
    # two identical ones-columns: fp32r matmul dst free size must be even,
    # so the softmax row sums are computed as N=2 (duplicate) columns
    onec_d = nc.inline_tensor(np.ones((128, 2), dtype=np.float32), "onec")

    with tile.TileContext(nc) as tc:
        with (
            tc.tile_pool(name="wp", bufs=1) as wp,
            tc.tile_pool(name="cp", bufs=1) as cp,
            tc.tile_pool(name="xtp", bufs=2) as xtp,
            tc.tile_pool(name="qtp", bufs=2) as qtp,
            tc.tile_pool(name="etp", bufs=2) as etp,
            tc.tile_pool(name="sp", bufs=4) as sp,
            tc.tile_pool(name="vp", bufs=4) as vp,
            tc.tile_pool(name="smp", bufs=8) as smp,
            tc.tile_pool(name="op", bufs=4) as op,
            tc.tile_pool(name="ps_qk", bufs=3, space=bass.MemorySpace.PSUM) as ps_qk,
            tc.tile_pool(name="ps_big", bufs=3, space=bass.MemorySpace.PSUM) as ps_big,
            tc.tile_pool(name="ps_sm", bufs=1, space=bass.MemorySpace.PSUM) as ps_sm,
            tc.tile_pool(name="ps_wm", bufs=1, space=bass.MemorySpace.PSUM) as ps_wm,
        ):
            # ---- resident constants (loaded later, on the sync queue, so
            # no second DMA ring has to be initialized at boot; they are
            # first needed ~16us in, well after the weight chunks) ----
            masks = cp.tile([128, 2, W], F32, tag="mask")
            onec = cp.tile([128, 2], F32R, tag="onec")
            ckc = cp.tile([128, 2 * nw], F32, tag="ckc")

            def load_consts():
                for kt in range(2):
                    nc.sync.dma_start(masks[:, kt, :], mask_d.ap()[kt])
                nc.sync.dma_start(onec[:], onec_d.ap().bitcast(F32R))
                nc.sync.dma_start(ckc[:], ckc_d.ap())

            # ---- resident weights: [p, ei, eo] = Wmat[ei*128+p, eo] ----
            msb = wp.tile([128, ET, E], BF16, tag="m", name="msb")
            nsb = wp.tile([128, ET, E], BF16, tag="n", name="nsb")
            m_r = m_d.ap().rearrange("(a p) n -> a p n", p=128)
            n_r = n_d.ap().rearrange("(a p) n -> a p n", p=128)

            def warm(col=0):
                # keep the PE activity monitor warm through the DMA-bound
                # phase: a tiny matmul per arriving chunk, paced by the DMA
                # itself (reads a slice of the chunk that just landed)
                wps = ps_wm.tile([128, 2], F32, tag="warm", name="warm")
                nc.tensor.matmul(
                    wps[:],
                    msb[:, 0, col : col + 128],
                    msb[:, 0, col : col + 2],
                    start=True,
                    stop=True,
                )

            xtr = xt_d.ap().rearrange("(a p) t -> a p t", p=128)
            xT_next = None
            for p in range(npair):
                ptok0 = p * PW

                # ---- xT[p, ei, t] for the pair (e-major, host-transposed) ----
                if p == 0:
                    xT = xtp.tile([128, ET, PW], BF16, tag="xT")
                    # interleave M column-chunks with per-ei chunks of the
                    # pair-0 xT load so the first q'-proj matmuls start as
                    # soon as (xT ei 0-1, M chunk 0) land
                    for eo in range(ET):
                        if eo < 4:
                            nc.sync.dma_start(
                                xT[:, 2 * eo : 2 * eo + 2, :],
                                xtr[2 * eo : 2 * eo + 2, :, 0:PW].transpose(
                                    [1, 0, 2]
                                ),
                            )
                        nc.sync.dma_start(
                            msb[:, :, eo * 128 : (eo + 1) * 128],
                            m_r[:, :, eo * 128 : (eo + 1) * 128].transpose([1, 0, 2]),
                        )
                        warm(eo * 128)
                    load_consts()
                else:
                    xT = xT_next

                # ---- q' projection for the pair -> q'T [e_out, t(512)] ----
                qT = qtp.tile([128, ET, PW], BF16, tag="qT")
                for eo in range(ET):
                    pp = ps_big.tile([128, PW], F32, tag="big")
                    for ei in range(ET):
                        nc.tensor.matmul(
                            pp[:],
                            msb[:, ei, eo * 128 : (eo + 1) * 128],
                            xT[:, ei, :],
                            start=(ei == 0),
                            stop=(ei == ET - 1),
                        )
                    nc.scalar.copy(qT[:, eo, :], pp[:])

                if p == 0:
                    # N chunk loads slot in behind the pair-0 q' matmuls
                    for half in range(2):
                        for eq in range(0, ET, 4):
                            nc.sync.dma_start(
                                nsb[:, eq : eq + 4, half * 512 : (half + 1) * 512],
                                n_r[
                                    eq : eq + 4, :, half * 512 : (half + 1) * 512
                                ].transpose([1, 0, 2]),
                            )

                # prefetch next pair's xT behind this pair's compute
                if p + 1 < npair:
                    xT_next = xtp.tile([128, ET, PW], BF16, tag="xT")
                    nc.sync.dma_start(
                        xT_next[:, :, :],
                        xtr[:, :, ptok0 + PW : ptok0 + 2 * PW].transpose([1, 0, 2]),
                    )

                for wi in range(2):
                    w = 2 * p + wi
                    tok0 = w * W
                    wt0 = wi * W  # token offset inside the pair tiles

                    # ---- transposed scores sT[k, q] + softmax ----
                    # causal: the kt=1 key block only serves queries q>=128;
                    # its block is computed 128 columns wide (q = 128+col).
                    expT = etp.tile([128, 2, W], F32R, tag="expT")
                    for kt in range(2):
                        qw = W if kt == 0 else 128
                        q0 = wt0 + kt * 128  # first query column needed
                        sc = ps_qk.tile([128, W], F32, tag="qk")
                        for ei in range(ET):
                            nc.tensor.matmul(
                                sc[:, 0:qw],
                                xT[:, ei, wt0 + kt * 128 : wt0 + (kt + 1) * 128],
                                qT[:, ei, q0 : q0 + qw],
                                start=(ei == 0),
                                stop=(ei == ET - 1),
                            )
                        s_sb = sp.tile([128, W], F32, tag="s")
                        nc.vector.tensor_add(
                            s_sb[:, 0:qw], sc[:, 0:qw], masks[:, kt, 0:qw]
                        )
                        # exp(SCALE*s + SCALE*ck[k]): per-key bias via ACT
                        nc.scalar.activation(
                            expT[:, kt, kt * 128 : kt * 128 + qw],
                            s_sb[:, 0:qw],
                            AF.Exp,
                            scale=SCALE,
                            bias=ckc[:, 2 * w + kt : 2 * w + kt + 1],
                        )

                    # ---- v' projection (token-major), fills PE during softmax ----
                    v_w = [
                        vp.tile([128, E], F32R, tag="v", name=f"v{kt}")
                        for kt in range(2)
                    ]
                    for kt in range(2):
                        for eoh in range(2):
                            pv = ps_big.tile([128, 512], F32, tag="big")
                            for ei in range(ET):
                                nc.tensor.matmul(
                                    pv[:],
                                    xT[:, ei, wt0 + kt * 128 : wt0 + (kt + 1) * 128],
                                    nsb[:, ei, eoh * 512 : (eoh + 1) * 512],
                                    start=(ei == 0),
                                    stop=(ei == ET - 1),
                                )
                            nc.vector.tensor_copy(
                                v_w[kt][:, eoh * 512 : (eoh + 1) * 512], pv[:]
                            )

                    # ---- softmax row sums (over k = partitions) ----
                    # qt=0 queries only attend to kt=0 keys (causal)
                    recs = []
                    for qt in range(2):
                        kts = (0,) if qt == 0 else (0, 1)
                        sm = ps_sm.tile([128, 2], F32, tag="sum")
                        for kt in kts:
                            nc.tensor.matmul(
                                sm[:],
                                expT[:, kt, qt * 128 : (qt + 1) * 128],
                                onec[:],
                                start=(kt == kts[0]),
                                stop=(kt == kts[-1]),
                            )
                        rec = smp.tile([128, 1], F32, tag="rec")
                        nc.vector.reciprocal(rec[:], sm[:, 0:1])
                        recs.append(rec)

                    # ---- out = attn @ v' (token-major), normalize in evac ----
                    for qt in range(2):
                        kts = (0,) if qt == 0 else (0, 1)
                        o_sb = op.tile([128, E], F32, tag="o")
                        for eoh in range(2):
                            po = ps_big.tile([128, 512], F32, tag="big")
                            for kt in kts:
                                nc.tensor.matmul(
                                    po[:],
                                    expT[:, kt, qt * 128 : (qt + 1) * 128],
                                    v_w[kt][:, eoh * 512 : (eoh + 1) * 512],
                                    start=(kt == kts[0]),
                                    stop=(kt == kts[-1]),
                                )
                            # normalize-evacuate on ACT; for the final
                            # window qt1 goes to the (now idle) DVE so both
                            # engines drain the tail concurrently
                            if qt == 0 or w < nw - 1:
                                nc.scalar.activation(
                                    o_sb[:, eoh * 512 : (eoh + 1) * 512],
                                    po[:],
                                    AF.Copy,
                                    scale=recs[qt][:],
                                )
                            else:
                                nc.vector.tensor_scalar_mul(
                                    o_sb[:, eoh * 512 : (eoh + 1) * 512],
                                    po[:],
                                    recs[qt][:],
                                )
                            # stream the final window's halves out as soon
                            # as each is evacuated; earlier windows store
                            # as one contiguous tile (fewer descriptors).
                            # Stores are emitted after the next pair's
                            # prefetch in sync-queue order, so they never
                            # head-of-line-block it.
                            if w == nw - 1:
                                nc.sync.dma_start(
                                    o_d.ap()[
                                        tok0 + qt * 128 : tok0 + (qt + 1) * 128,
                                        eoh * 512 : (eoh + 1) * 512,
                                    ],
                                    o_sb[:, eoh * 512 : (eoh + 1) * 512],
                                )
                        if w < nw - 1:
                            nc.sync.dma_start(
                                o_d.ap()[tok0 + qt * 128 : tok0 + (qt + 1) * 128, :],
                                o_sb[:],
                            )

    nc.compile()
    return nc


_NC_CACHE = {}


def _get_nc(nw=NW):
    if nw not in _NC_CACHE:
        _NC_CACHE[nw] = build_nc(nw)
    return _NC_CACHE[nw]


def prepare(x, Wq, bq, Wk, bk, Wv, bv, Wo, bo):
    """Host-side precompute: per-core input maps + host residual terms."""
    x = np.asarray(x, dtype=np.float32)
    B, S, _ = x.shape
    x_flat = np.ascontiguousarray(x.reshape(B * S, E))
    t_core = B * S // N_CORES
    assert t_core == T

    f64 = np.float64
    Wq64, Wk64 = np.asarray(Wq, f64), np.asarray(Wk, f64)
    Wv64, Wo64 = np.asarray(Wv, f64), np.asarray(Wo, f64)
    bq64, bk64 = np.asarray(bq, f64), np.asarray(bk, f64)
    bv64, bo64 = np.asarray(bv, f64), np.asarray(bo, f64)

    M = np.ascontiguousarray((Wq64.T @ Wk64).astype(NP_BF16))
    N = np.ascontiguousarray((Wv64.T @ Wo64.T).astype(NP_BF16))
    ck = (x_flat.astype(f64) @ (Wk64.T @ bq64)) * SCALE  # [T_total]
    orow = (bv64 @ Wo64.T + bo64).astype(np.float32)  # [E]
    xt_full = x_flat.T.astype(NP_BF16)

    common = {"m": M, "n": N}
    in_maps = [
        {
            "xt": np.ascontiguousarray(xt_full[:, i * t_core : (i + 1) * t_core]),
            # ck columns: [128, 2*nw], one column per 128-token chunk
            "ckc": np.ascontiguousarray(
                ck[i * t_core : (i + 1) * t_core]
                .astype(np.float32)
                .reshape(2 * NW, 128)
                .T
            ),
            **common,
        }
        for i in range(N_CORES)
    ]
    return in_maps, orow, x_flat, (B, S)


def kernel(x, Wq, bq, Wk, bk, Wv, bv, Wo, bo):
    in_maps, orow, x_flat, (B, S) = prepare(x, Wq, bq, Wk, bk, Wv, bv, Wo, bo)
    nc = _get_nc()
    res = run_bass_kernel_spmd(nc, in_maps, core_ids=list(range(N_CORES)))
    out = np.concatenate([res.results[i]["o"] for i in range(N_CORES)], axis=0)
    out += orow[None, :]
    out += x_flat
    return out.reshape(B, S, E).astype(np.float32)
